# revision 36
# baseline (speedup 1.0000x reference)
"""DeepSeekMoE (BitNet-quantized) Trainium2 kernel.

Strategy (8 NeuronCores, SPMD):
  - Host: rmsnorm + activation quant + router (bf16 logits, exact replication
    of the reference's routing) + top-k dispatch. Weights are ternary-quantized
    on host (BitNet b1.58) and shipped as fp8e4 {-1,0,+1} matrices; activations
    are shipped as int8-valued bf16.  All heavy matmuls then run EXACTLY on
    the PE (integer arithmetic, fp32 accumulation is exact).
  - Core i: routed expert i on its dispatched tokens (capacity 512 = 4 tiles
    of 128; the handful of tokens beyond capacity are computed exactly on the
    host), plus shared expert i//4 on token block i%4 (512 tokens).
  - Device pipeline per 128-token tile: fc1 (PE, weights moving) -> silu +
    int8 requant (scalar+DVE) -> tiled transpose via the DMA xbar (no PE
    cycles) -> fc2 (PE) -> raw psum copied out; the per-token scale
    gate*sc2*absmax/127^2 is applied on the host from the shipped absmaxes,
    so the PE runs matmuls only (~91% occupancy, bf16 roofline).
  - Host: gathers per-core outputs, scatter-adds routed contributions.
"""

import numpy as np
import ml_dtypes

BF16 = ml_dtypes.bfloat16
F8 = ml_dtypes.float8_e4m3
F32 = np.float32

P = 128
D_ = 1024
F_ = 2048
E_ = 8
T_ = 2048
NCORES = 8
C_ROUT = 512  # routed-token capacity per expert; overflow handled on host
              # (seed-0 counts are 472..542, so <=30 overflow rows/expert)
T_SH = 512    # shared-expert token block per core
MAGIC = float(1.5 * 2 ** 23)  # round-to-nearest-even magic constant (f32)

TRACE = False
_LAST_RESULTS = None
_NC_CACHE = None


# ----------------------------------------------------------------------------
# host-side math (replicates reference.py numerics)
# ----------------------------------------------------------------------------

def _rmsnorm(x2d, w):
    ms = np.mean(x2d * x2d, axis=-1, dtype=np.float32, keepdims=True) + F32(1e-6)
    return (x2d * (F32(1.0) / np.sqrt(ms)) * w).astype(np.float32)


def _quant_a(h):
    # returns integer levels n in [-128,127] (f32) and scale s with q = n / s
    mx = np.maximum(np.abs(h).max(axis=-1), F32(1e-5)).astype(np.float32)
    s = (F32(127.0) / mx).astype(np.float32)
    n = np.clip(np.round(h * s[:, None]), -128.0, 127.0).astype(np.float32)
    return n, s


def _quant_w(w):
    # per-matrix ternary quant; returns ternary (f32 {-1,0,1}) and scale
    scale = F32(np.mean(np.abs(w), dtype=np.float32) + F32(1e-8))
    t = np.clip(np.round(w / scale), -1.0, 1.0).astype(np.float32)
    return t, scale


def _route(h, router_w, top_k):
    hb = h.astype(BF16).astype(np.float32)
    rb = router_w.astype(BF16).astype(np.float32)
    logits = (hb @ rb.T).astype(BF16).astype(np.float32)
    m = logits.max(-1, keepdims=True)
    p = np.exp(logits - m)
    p /= p.sum(-1, keepdims=True)
    order = np.argsort(-p, axis=-1, kind="stable")
    idx = order[:, :top_k]
    g = np.take_along_axis(p, idx, -1)
    g = (g / g.sum(-1, keepdims=True)).astype(np.float32)
    return idx, g


def _silu(x):
    return x / (1.0 + np.exp(-x))


def _expert_mlp_rows(nq, s1, t1, sc1, t2, sc2):
    # exact numpy replication of one expert on quantized rows (fallback path)
    a = (nq / s1[:, None]) @ (t1 * sc1)
    a = _silu(a).astype(np.float32)
    n2, s2 = _quant_a(a)
    return ((n2 / s2[:, None]) @ (t2 * sc2)).astype(np.float32)


# ----------------------------------------------------------------------------
# device kernel
# ----------------------------------------------------------------------------

def _build_nc(loop_n=None):
    from concourse import bacc, mybir, tile, masks

    dt = mybir.dt
    AF = mybir.ActivationFunctionType
    ALU = mybir.AluOpType
    AX = mybir.AxisListType

    nc = bacc.Bacc("TRN2", target_bir_lowering=False, debug=False,
                   num_devices=NCORES)

    def din(name, shape, dtype):
        return nc.dram_tensor(name, shape, dtype, kind="ExternalInput").ap()

    KD = D_ // P   # 8  fc1 contraction tiles
    KF = F_ // P   # 16 fc2 contraction tiles
    NF = F_ // 512  # 4 fc1 output tiles
    ND = D_ // 512  # 2 fc2 output tiles
    MTR = C_ROUT // P
    MTS = T_SH // P

    # activations packed [mt*P + d_sub, k*P + t] so each token-tile slice is
    # one DMA with 2KB contiguous runs (d-partition-major within a tile)
    a_r = din("a_r", [MTR * P, KD * P], dt.bfloat16)
    a_s = din("a_s", [MTS * P, KD * P], dt.bfloat16)
    w1r = din("w1r", [D_, F_], dt.float8e4)
    w2r = din("w2r", [F_, D_], dt.float8e4)
    w1s = din("w1s", [D_, F_], dt.float8e4)
    w2s = din("w2s", [F_, D_], dt.float8e4)
    # per-token fc1 psum scale (sc1 / s1[token]) per unit
    sc_r = din("sc_r", [P, MTR], dt.float32)
    sc_s = din("sc_s", [P, MTS], dt.float32)

    # raw (unscaled) fc2 accumulators; host applies gate*sc2*mxc/127
    out_r = nc.dram_tensor("out_r", [C_ROUT, D_], dt.float32,
                           kind="ExternalOutput").ap()
    out_s = nc.dram_tensor("out_s", [T_SH, D_], dt.float32,
                           kind="ExternalOutput").ap()
    # fc1 per-token absmax (clipped at 1e-5), routed cols then shared cols
    out_v = nc.dram_tensor("out_v", [P, MTR + MTS], dt.float32,
                           kind="ExternalOutput").ap()

    import contextlib

    with tile.TileContext(nc) as tc:
        with (
            tc.tile_pool(name="wpool", bufs=1) as wpool,
            tc.tile_pool(name="apool", bufs=1) as apool,
            tc.tile_pool(name="spool", bufs=1) as spool,
            tc.tile_pool(name="work", bufs=2) as work,
            tc.tile_pool(name="small", bufs=4) as small,
            tc.tile_pool(name="pp1", bufs=4, space="PSUM") as pp1,
            tc.tile_pool(name="pp2", bufs=2, space="PSUM") as pp2,
            (tc.For_i(0, loop_n, 1,
                      hint_engines=(mybir.EngineType.PE,
                                    mybir.EngineType.DVE,
                                    mybir.EngineType.Activation,
                                    mybir.EngineType.SP))
             if loop_n is not None else contextlib.nullcontext()),
        ):
            sc_r_sb = spool.tile([P, MTR], dt.float32, tag="sc_r_sb")
            sc_s_sb = spool.tile([P, MTS], dt.float32, tag="sc_s_sb")
            # per-token fc1 absmax, written per tile, shipped at the end
            mx_all = spool.tile([P, MTR + MTS], dt.float32, tag="mx_all")

            # Single merged SBUF tiles per tensor; one big strided DMA each
            # -- the ~0.6us serialized HWDGE cost is per DMA instruction, so
            # fewer/larger transfers keep the queue short. The activation
            # tiles are [P, mt, k, t] so a token-tile slice is contiguous.
            at_r_t = apool.tile([P, MTR, KD, P], dt.bfloat16, tag="at_r")
            at_s_t = apool.tile([P, MTS, KD, P], dt.bfloat16, tag="at_s")
            w1r_t = wpool.tile([P, KD, F_], dt.float8e4, tag="w1r")
            w2r_t = wpool.tile([P, KF, D_], dt.float8e4, tag="w2r")
            w1s_t = wpool.tile([P, KD, F_], dt.float8e4, tag="w1s")
            w2s_t = wpool.tile([P, KF, D_], dt.float8e4, tag="w2s")

            a_r3 = a_r.rearrange("(mt p) c -> p mt c", p=P)
            a_s3 = a_s.rearrange("(mt p) c -> p mt c", p=P)
            w1r3 = w1r.rearrange("(k p) f -> p k f", p=P)
            w2r3 = w2r.rearrange("(k p) d -> p k d", p=P)
            w1s3 = w1s.rearrange("(k p) f -> p k f", p=P)
            w2s3 = w2s.rearrange("(k p) d -> p k d", p=P)

            def at_dma(dst, src, mt0, mt1):
                nc.sync.dma_start(
                    dst[:, mt0:mt1, :, :],
                    src[:, mt0:mt1, :].rearrange("p m (k t) -> p m k t", t=P))

            # DMA emission order follows first-use on the PE: the k=0 slice
            # of token-tile-0 (32KB, gates the first ldweights) and the k=0
            # weight chunk land first so fc1 of tile 0 starts immediately;
            # the rest streams behind it in consumption order.
            nc.sync.dma_start(
                at_r_t[:, 0:1, 0:1, :],
                a_r3[:, 0:1, 0:P].rearrange("p m (k t) -> p m k t", t=P))
            nc.sync.dma_start(w1r_t[:, 0:1, 0:512], w1r3[:, 0:1, 0:512])
            nc.sync.dma_start(
                at_r_t[:, 0:1, 1:KD, :],
                a_r3[:, 0:1, P:].rearrange("p m (k t) -> p m k t", t=P))
            nc.sync.dma_start(w1r_t[:, 1:4, 0:512], w1r3[:, 1:4, 0:512])
            nc.sync.dma_start(w1r_t[:, 4:KD, 0:512], w1r3[:, 4:KD, 0:512])
            nc.sync.dma_start(sc_r_sb[:], sc_r[:])
            nc.sync.dma_start(w1r_t[:, :, 512:1024], w1r3[:, :, 512:1024])
            nc.sync.dma_start(w1r_t[:, :, 1024:1536], w1r3[:, :, 1024:1536])
            nc.sync.dma_start(w1r_t[:, :, 1536:], w1r3[:, :, 1536:])
            at_dma(at_r_t, a_r3, 1, 2)
            at_dma(at_r_t, a_r3, 2, MTR)
            nc.sync.dma_start(w2r_t[:, 0:KF // 2, :], w2r3[:, 0:KF // 2, :])
            nc.sync.dma_start(w2r_t[:, KF // 2:, :], w2r3[:, KF // 2:, :])
            at_dma(at_s_t, a_s3, 0, MTS)
            nc.sync.dma_start(w1s_t[:], w1s3[:])
            nc.sync.dma_start(sc_s_sb[:], sc_s[:])
            nc.sync.dma_start(w2s_t[:], w2s3[:])

            # flat list of M-tiles across both expert units
            tiles = [(at_r_t, w1r_t, w2r_t, sc_r_sb[:, mt:mt + 1],
                      mt, out_r, mt) for mt in range(MTR)]
            tiles += [(at_s_t, w1s_t, w2s_t, sc_s_sb[:, mt:mt + 1],
                       MTR + mt, out_s, mt) for mt in range(MTS)]

            def front(at, w1t, cs1_c, slot, mt):
                """fc1 + silu + quant -> returns n2; absmax lands in
                mx_all[:, slot] for the end-of-kernel ship-out."""
                asl = work.tile([P, F_], dt.float32, tag="asl", name="asl")
                mx4 = small.tile([P, NF], dt.float32, tag="mx4", name="mx4")
                for n in range(NF):
                    ps1 = pp1.tile([P, 512], dt.float32, tag="ps1", name="ps1")
                    for k in range(KD):
                        nc.tensor.matmul(
                            ps1[:],
                            at[:, mt, k, :],
                            w1t[:, k, n * 512:(n + 1) * 512],
                            start=(k == 0), stop=(k == KD - 1))
                    # a = silu(psum * cs1[token])
                    nc.scalar.activation(
                        asl[:, n * 512:(n + 1) * 512], ps1[:], AF.Silu,
                        scale=cs1_c)
                    # per-chunk absmax (hides under the next chunk's fc1)
                    nc.vector.tensor_reduce(
                        mx4[:, n:n + 1], asl[:, n * 512:(n + 1) * 512],
                        AX.X, ALU.max, apply_absolute_value=True)
                # combine chunk maxes; clip to 1e-5
                mxc = mx_all[:, slot:slot + 1]
                nc.vector.tensor_reduce(mxc, mx4[:], AX.X, ALU.max)
                nc.vector.tensor_scalar_max(mxc, mxc, 1e-5)
                r127 = small.tile([P, 1], dt.float32, tag="r127", name="r127")
                nc.vector.reciprocal(r127[:], mxc)
                nc.vector.tensor_scalar_mul(r127[:], r127[:], 127.0)
                # n2 = clip(round(a * 127/max), -128, 127), in-place round
                nc.vector.tensor_scalar(asl[:], asl[:], r127[:], MAGIC,
                                        ALU.mult, ALU.add)
                nc.vector.tensor_scalar(asl[:], asl[:], MAGIC, -128.0,
                                        ALU.subtract, ALU.max)
                n2 = work.tile([P, F_], dt.bfloat16, tag="n2", name="n2",
                               bufs=3)
                nc.vector.tensor_scalar(n2[:], asl[:], 127.0, None, ALU.min)
                return n2

            def trans(n2):
                """Tiled transpose of the quantized fc1 output via the DMA
                xbar: n2T[p, k, t] = n2[t, k*128+p]. Runs on the DMA engines,
                so the PE does matmuls only."""
                n2T = work.tile([P, KF, P], dt.bfloat16, tag="n2T",
                                name="n2T")
                nc.sync.dma_start_transpose(n2T[:], n2[:])
                return n2T

            def fc2(n2T, w2t, out_d, mt, last=False):
                """fc2; raw accumulators are copied out on the DVE and the
                per-token scale is applied on the host, keeping the
                end-of-kernel tail short. The last tile tapers its output
                chunks (512/384/128) so the final copy+store is small."""
                outsb = work.tile([P, D_], dt.float32, tag="outsb",
                                  name="outsb")
                chunks = [(0, 512), (512, 384), (896, 128)] if last else [
                    (0, 512), (512, 512)]
                for c0, cw in chunks:
                    ps2 = pp2.tile([P, 512], dt.float32, tag="ps2", name="ps2")
                    for k in range(KF):
                        nc.tensor.matmul(
                            ps2[:, 0:cw],
                            n2T[:, k, :],
                            w2t[:, k, c0:c0 + cw],
                            start=(k == 0), stop=(k == KF - 1))
                    nc.vector.tensor_copy(
                        outsb[:, c0:c0 + cw], ps2[:, 0:cw])
                    nc.sync.dma_start(
                        out_d[mt * P:(mt + 1) * P, c0:c0 + cw],
                        outsb[:, c0:c0 + cw])

            # 3-stage software pipeline: per iteration the PE runs
            # fc1(i) -> trans(i-1) -> fc2(i-2), so the DVE quant chain of
            # tile i-1 and the scalar copy of tile i-1's transpose are both
            # hidden under PE matmul work, including at the drain.
            trans_q = []   # awaiting transpose
            fc2_q = []     # awaiting fc2
            for i, (at, w1t, w2t, cs1_c, slot, out_d, mt) in enumerate(tiles):
                n2 = front(at, w1t, cs1_c, slot, mt)
                trans_q.append((n2, w2t, out_d, mt))
                if len(trans_q) > 1:
                    tn2, tw2t, tout_d, tmt = trans_q.pop(0)
                    fc2_q.append((trans(tn2), tw2t, tout_d, tmt))
                if len(fc2_q) > 1:
                    fc2(*fc2_q.pop(0))
            # ship the per-token absmaxes once the last front has run
            nc.sync.dma_start(out_v[:], mx_all[:])
            while trans_q:
                tn2, tw2t, tout_d, tmt = trans_q.pop(0)
                fc2_q.append((trans(tn2), tw2t, tout_d, tmt))
                if fc2_q:
                    fc2(*fc2_q.pop(0))
            while fc2_q:
                p = fc2_q.pop(0)
                fc2(*p, last=not fc2_q)

    nc.compile()
    return nc


def _get_nc():
    global _NC_CACHE
    if _NC_CACHE is None:
        _NC_CACHE = _build_nc()
    return _NC_CACHE


# ----------------------------------------------------------------------------
# entry point
# ----------------------------------------------------------------------------

def _prepare(x, rms_w, w1_shared, w2_shared, w1_routed, w2_routed, router_w,
             top_k):
    x = np.asarray(x)
    B, S, D = x.shape
    T = B * S
    E = np.asarray(router_w).shape[0]
    SH = np.asarray(w1_shared).shape[0]
    k_ = int(top_k)
    assert (T, D, E, SH) == (T_, D_, E_, 2) and k_ == 2

    h = _rmsnorm(x.reshape(T, D).astype(np.float32), np.asarray(rms_w))
    n1, s1 = _quant_a(h)
    idx, g = _route(h, np.asarray(router_w), k_)

    # ternary weights + scales
    t1r, sc1r, t2r, sc2r = [], [], [], []
    for e in range(E):
        t, s = _quant_w(np.asarray(w1_routed)[e]); t1r.append(t); sc1r.append(s)
        t, s = _quant_w(np.asarray(w2_routed)[e]); t2r.append(t); sc2r.append(s)
    t1s, sc1s_, t2s, sc2s_ = [], [], [], []
    for e in range(SH):
        t, s = _quant_w(np.asarray(w1_shared)[e]); t1s.append(t); sc1s_.append(s)
        t, s = _quant_w(np.asarray(w2_shared)[e]); t2s.append(t); sc2s_.append(s)

    n1_bf = n1.astype(BF16)

    # dispatch: token lists per expert (ascending order)
    tok_lists = [np.where((idx == e).any(axis=1))[0] for e in range(E)]
    gate_of = np.zeros((T, E), dtype=np.float32)
    for slot in range(k_):
        gate_of[np.arange(T), idx[:, slot]] += g[:, slot]

    def pack_act(rows):
        # [T, D] -> [mt, d_sub, k, t] -> [(mt d_sub), (k t)]
        mt = rows.shape[0] // P
        return np.ascontiguousarray(
            rows.reshape(mt, P, D_ // P, P).transpose(0, 3, 2, 1)
                .reshape(mt * P, D_))

    in_maps = []
    for i in range(NCORES):
        toks = tok_lists[i][:C_ROUT]
        nct = len(toks)
        a_r = np.zeros((C_ROUT, D_), dtype=BF16)
        a_r[:nct] = n1_bf[toks]
        cs1_v = np.zeros(C_ROUT, dtype=np.float32)
        cs1_v[:nct] = sc1r[i] / s1[toks]

        sh, blk = i // 4, i % 4
        btok = slice(blk * T_SH, (blk + 1) * T_SH)
        a_s = n1_bf[btok]
        cs1s_v = (sc1s_[sh] / s1[btok]).astype(np.float32)

        in_maps.append({
            "a_r": pack_act(a_r),
            "a_s": pack_act(np.ascontiguousarray(a_s)),
            "w1r": t1r[i].astype(F8),
            "w2r": t2r[i].astype(F8),
            "w1s": t1s[sh].astype(F8),
            "w2s": t2s[sh].astype(F8),
            "sc_r": np.ascontiguousarray(cs1_v.reshape(-1, P).T),
            "sc_s": np.ascontiguousarray(cs1s_v.reshape(-1, P).T),
        })

    meta = {
        "B": B, "S": S, "T": T,
        "tok_lists": tok_lists, "gate_of": gate_of,
        "n1": n1, "s1": s1, "t1r": t1r, "sc1r": sc1r,
        "t2r": t2r, "sc2r": sc2r, "sc2s": sc2s_,
    }
    return in_maps, meta


def _assemble(results, meta):
    T = meta["T"]
    tok_lists = meta["tok_lists"]
    MTR = C_ROUT // P
    acc = np.zeros((T, D_), dtype=np.float32)
    for i in range(NCORES):
        om = results[i]
        sh, blk = i // 4, i % 4
        # out_v columns: routed tiles then shared tiles; token = mt*P + p
        mx = om["out_v"]
        mx_r = mx[:, :MTR].T.reshape(-1)
        mx_s = mx[:, MTR:].T.reshape(-1)
        toks = tok_lists[i][:C_ROUT]
        nct = len(toks)
        v_r = (meta["gate_of"][toks, i] * meta["sc2r"][i] / F32(127.0)
               * mx_r[:nct]).astype(np.float32)
        np.add.at(acc, toks, om["out_r"][:nct] * v_r[:, None])
        v_s = (meta["sc2s"][sh] / F32(127.0) * mx_s).astype(np.float32)
        acc[blk * T_SH:(blk + 1) * T_SH] += om["out_s"] * v_s[:, None]
        # capacity-overflow fallback (exact replication on host)
        if len(tok_lists[i]) > C_ROUT:
            extra = tok_lists[i][C_ROUT:]
            out_e = _expert_mlp_rows(
                meta["n1"][extra], meta["s1"][extra], meta["t1r"][i],
                meta["sc1r"][i], meta["t2r"][i], meta["sc2r"][i])
            acc[extra] += meta["gate_of"][extra, i][:, None] * out_e
    return acc.reshape(meta["B"], meta["S"], D_).astype(np.float32)


def kernel(x, rms_w, w1_shared, w2_shared, w1_routed, w2_routed, router_w,
           top_k):
    global _LAST_RESULTS
    in_maps, meta = _prepare(x, rms_w, w1_shared, w2_shared, w1_routed,
                             w2_routed, router_w, top_k)
    from concourse import bass_utils
    nc = _get_nc()
    res = bass_utils.run_bass_kernel_spmd(
        nc, in_maps, core_ids=list(range(NCORES)), trace=TRACE)
    _LAST_RESULTS = res
    return _assemble(res.results, meta)



# revision 39
# speedup vs baseline: 1.1408x; 1.1408x over previous
"""DeepSeekMoE (BitNet-quantized) Trainium2 kernel.

Strategy (8 NeuronCores, SPMD):
  - Host: rmsnorm + activation quant + router (bf16 logits, exact replication
    of the reference's routing) + top-k dispatch. Weights are ternary-quantized
    on host (BitNet b1.58) and shipped as fp8e4 {-1,0,+1} matrices; activations
    are shipped as int8-valued bf16.  All heavy matmuls then run EXACTLY on
    the PE (integer arithmetic, fp32 accumulation is exact).
  - Core i: routed expert i on its dispatched tokens (capacity 512 = 4 tiles
    of 128; the handful of tokens beyond capacity are computed exactly on the
    host), plus shared expert i//4 on token block i%4 (512 tokens).
  - Device pipeline per 128-token tile: fc1 (PE, weights moving) -> silu +
    int8 requant (scalar+DVE) -> tiled transpose via the DMA xbar (no PE
    cycles) -> fc2 (PE) -> raw psum copied out; the per-token scale
    gate*sc2*absmax/127^2 is applied on the host from the shipped absmaxes,
    so the PE runs matmuls only (~91% occupancy, bf16 roofline).
  - Host: gathers per-core outputs, scatter-adds routed contributions.
"""

import numpy as np
import ml_dtypes

BF16 = ml_dtypes.bfloat16
F8 = ml_dtypes.float8_e4m3
F32 = np.float32

P = 128
D_ = 1024
F_ = 2048
E_ = 8
T_ = 2048
NCORES = 8
C_ROUT = 512  # routed-token capacity per expert; overflow handled on host
              # (seed-0 counts are 472..542, so <=30 overflow rows/expert)
T_SH = 512    # shared-expert token block per core
MAGIC = float(1.5 * 2 ** 23)  # round-to-nearest-even magic constant (f32)

TRACE = False
_LAST_RESULTS = None
_NC_CACHE = None


# ----------------------------------------------------------------------------
# host-side math (replicates reference.py numerics)
# ----------------------------------------------------------------------------

def _rmsnorm(x2d, w):
    ms = np.mean(x2d * x2d, axis=-1, dtype=np.float32, keepdims=True) + F32(1e-6)
    return (x2d * (F32(1.0) / np.sqrt(ms)) * w).astype(np.float32)


def _quant_a(h):
    # returns integer levels n in [-128,127] (f32) and scale s with q = n / s
    mx = np.maximum(np.abs(h).max(axis=-1), F32(1e-5)).astype(np.float32)
    s = (F32(127.0) / mx).astype(np.float32)
    n = np.clip(np.round(h * s[:, None]), -128.0, 127.0).astype(np.float32)
    return n, s


def _quant_w(w):
    # per-matrix ternary quant; returns ternary (f32 {-1,0,1}) and scale
    scale = F32(np.mean(np.abs(w), dtype=np.float32) + F32(1e-8))
    t = np.clip(np.round(w / scale), -1.0, 1.0).astype(np.float32)
    return t, scale


def _route(h, router_w, top_k):
    hb = h.astype(BF16).astype(np.float32)
    rb = router_w.astype(BF16).astype(np.float32)
    logits = (hb @ rb.T).astype(BF16).astype(np.float32)
    m = logits.max(-1, keepdims=True)
    p = np.exp(logits - m)
    p /= p.sum(-1, keepdims=True)
    order = np.argsort(-p, axis=-1, kind="stable")
    idx = order[:, :top_k]
    g = np.take_along_axis(p, idx, -1)
    g = (g / g.sum(-1, keepdims=True)).astype(np.float32)
    return idx, g


def _silu(x):
    return x / (1.0 + np.exp(-x))


def _expert_mlp_rows(nq, s1, t1, sc1, t2, sc2):
    # exact numpy replication of one expert on quantized rows (fallback path)
    a = (nq / s1[:, None]) @ (t1 * sc1)
    a = _silu(a).astype(np.float32)
    n2, s2 = _quant_a(a)
    return ((n2 / s2[:, None]) @ (t2 * sc2)).astype(np.float32)


# ----------------------------------------------------------------------------
# device kernel
# ----------------------------------------------------------------------------

def _build_nc(loop_n=None):
    from concourse import bacc, mybir, tile

    dt = mybir.dt
    AF = mybir.ActivationFunctionType
    ALU = mybir.AluOpType
    AX = mybir.AxisListType

    nc = bacc.Bacc("TRN2", target_bir_lowering=False, debug=False,
                   num_devices=NCORES)

    def din(name, shape, dtype):
        return nc.dram_tensor(name, shape, dtype, kind="ExternalInput").ap()

    KD = D_ // P   # 8  fc1 contraction tiles
    KF = F_ // P   # 16 fc2 contraction tiles
    NF = F_ // 512  # 4 fc1 output tiles
    ND = D_ // 512  # 2 fc2 output tiles
    MTR = C_ROUT // P
    MTS = T_SH // P

    # activations packed [mt*P + d_sub, k*P + t] so each token-tile slice is
    # one DMA with 2KB contiguous runs (d-partition-major within a tile)
    a_r = din("a_r", [MTR * P, KD * P], dt.bfloat16)
    a_s = din("a_s", [MTS * P, KD * P], dt.bfloat16)
    w1r = din("w1r", [D_, F_], dt.float8e4)
    w2r = din("w2r", [F_, D_], dt.float8e4)
    w1s = din("w1s", [D_, F_], dt.float8e4)
    w2s = din("w2s", [F_, D_], dt.float8e4)
    # per-token fc1 psum scale (sc1 / s1[token]) per unit
    sc_r = din("sc_r", [P, MTR], dt.float32)
    sc_s = din("sc_s", [P, MTS], dt.float32)

    # raw (unscaled) fc2 accumulators; host applies gate*sc2*mxc/127
    out_r = nc.dram_tensor("out_r", [C_ROUT, D_], dt.float32,
                           kind="ExternalOutput").ap()
    out_s = nc.dram_tensor("out_s", [T_SH, D_], dt.float32,
                           kind="ExternalOutput").ap()
    # fc1 per-token absmax (clipped at 1e-5), routed cols then shared cols
    out_v = nc.dram_tensor("out_v", [P, MTR + MTS], dt.float32,
                           kind="ExternalOutput").ap()

    import contextlib

    with tile.TileContext(nc) as tc:
        with (
            tc.tile_pool(name="wpool", bufs=1) as wpool,
            tc.tile_pool(name="apool", bufs=1) as apool,
            tc.tile_pool(name="spool", bufs=1) as spool,
            tc.tile_pool(name="work", bufs=2) as work,
            tc.tile_pool(name="small", bufs=4) as small,
            tc.tile_pool(name="pp1", bufs=4, space="PSUM") as pp1,
            tc.tile_pool(name="pp2", bufs=2, space="PSUM") as pp2,
            (tc.For_i(0, loop_n, 1,
                      hint_engines=(mybir.EngineType.PE,
                                    mybir.EngineType.DVE,
                                    mybir.EngineType.Activation,
                                    mybir.EngineType.SP))
             if loop_n is not None else contextlib.nullcontext()),
        ):
            sc_r_sb = spool.tile([P, MTR], dt.float32, tag="sc_r_sb")
            sc_s_sb = spool.tile([P, MTS], dt.float32, tag="sc_s_sb")
            # per-token fc1 absmax, written per tile, shipped at the end
            mx_all = spool.tile([P, MTR + MTS], dt.float32, tag="mx_all")

            # Single merged SBUF tiles per tensor; one big strided DMA each
            # -- the ~0.6us serialized HWDGE cost is per DMA instruction, so
            # fewer/larger transfers keep the queue short. The activation
            # tiles are [P, mt, k, t] so a token-tile slice is contiguous.
            at_r_t = apool.tile([P, MTR, KD, P], dt.bfloat16, tag="at_r")
            at_s_t = apool.tile([P, MTS, KD, P], dt.bfloat16, tag="at_s")
            w1r_t = wpool.tile([P, KD, F_], dt.float8e4, tag="w1r")
            w2r_t = wpool.tile([P, KF, D_], dt.float8e4, tag="w2r")
            w1s_t = wpool.tile([P, KD, F_], dt.float8e4, tag="w1s")
            w2s_t = wpool.tile([P, KF, D_], dt.float8e4, tag="w2s")

            a_r3 = a_r.rearrange("(mt p) c -> p mt c", p=P)
            a_s3 = a_s.rearrange("(mt p) c -> p mt c", p=P)
            w1r3 = w1r.rearrange("(k p) f -> p k f", p=P)
            w2r3 = w2r.rearrange("(k p) d -> p k d", p=P)
            w1s3 = w1s.rearrange("(k p) f -> p k f", p=P)
            w2s3 = w2s.rearrange("(k p) d -> p k d", p=P)

            def at_dma(dst, src, mt0, mt1):
                nc.sync.dma_start(
                    dst[:, mt0:mt1, :, :],
                    src[:, mt0:mt1, :].rearrange("p m (k t) -> p m k t", t=P))

            # DMA emission order follows first-use on the PE: the k=0 slice
            # of token-tile-0 (32KB, gates the first ldweights) and the k=0
            # weight chunk land first so fc1 of tile 0 starts immediately;
            # the rest streams behind it in consumption order.
            nc.sync.dma_start(
                at_r_t[:, 0:1, 0:1, :],
                a_r3[:, 0:1, 0:P].rearrange("p m (k t) -> p m k t", t=P))
            nc.sync.dma_start(w1r_t[:, 0:1, 0:512], w1r3[:, 0:1, 0:512])
            nc.sync.dma_start(
                at_r_t[:, 0:1, 1:KD, :],
                a_r3[:, 0:1, P:].rearrange("p m (k t) -> p m k t", t=P))
            nc.sync.dma_start(w1r_t[:, 1:4, 0:512], w1r3[:, 1:4, 0:512])
            nc.sync.dma_start(w1r_t[:, 4:KD, 0:512], w1r3[:, 4:KD, 0:512])
            nc.sync.dma_start(sc_r_sb[:], sc_r[:])
            nc.sync.dma_start(w1r_t[:, :, 512:1024], w1r3[:, :, 512:1024])
            nc.sync.dma_start(w1r_t[:, :, 1024:1536], w1r3[:, :, 1024:1536])
            nc.sync.dma_start(w1r_t[:, :, 1536:], w1r3[:, :, 1536:])
            at_dma(at_r_t, a_r3, 1, 2)
            at_dma(at_r_t, a_r3, 2, MTR)
            nc.sync.dma_start(w2r_t[:, 0:KF // 2, :], w2r3[:, 0:KF // 2, :])
            nc.sync.dma_start(w2r_t[:, KF // 2:, :], w2r3[:, KF // 2:, :])
            at_dma(at_s_t, a_s3, 0, MTS)
            nc.sync.dma_start(w1s_t[:], w1s3[:])
            nc.sync.dma_start(sc_s_sb[:], sc_s[:])
            nc.sync.dma_start(w2s_t[:], w2s3[:])

            # flat list of M-tiles across both expert units
            tiles = [(at_r_t, w1r_t, w2r_t, sc_r_sb[:, mt:mt + 1],
                      mt, out_r, mt) for mt in range(MTR)]
            tiles += [(at_s_t, w1s_t, w2s_t, sc_s_sb[:, mt:mt + 1],
                       MTR + mt, out_s, mt) for mt in range(MTS)]

            def front(at, w1t, cs1_c, slot, mt):
                """fc1 + silu + quant -> returns n2; absmax lands in
                mx_all[:, slot] for the end-of-kernel ship-out."""
                asl = work.tile([P, F_], dt.float32, tag="asl", name="asl")
                mx4 = small.tile([P, NF], dt.float32, tag="mx4", name="mx4")
                for n in range(NF):
                    ps1 = pp1.tile([P, 512], dt.float32, tag="ps1", name="ps1")
                    for k in range(KD):
                        nc.tensor.matmul(
                            ps1[:],
                            at[:, mt, k, :],
                            w1t[:, k, n * 512:(n + 1) * 512],
                            start=(k == 0), stop=(k == KD - 1))
                    # a = silu(psum * cs1[token])
                    nc.scalar.activation(
                        asl[:, n * 512:(n + 1) * 512], ps1[:], AF.Silu,
                        scale=cs1_c)
                    # per-chunk absmax (hides under the next chunk's fc1)
                    nc.vector.tensor_reduce(
                        mx4[:, n:n + 1], asl[:, n * 512:(n + 1) * 512],
                        AX.X, ALU.max, apply_absolute_value=True)
                # combine chunk maxes; clip to 1e-5
                mxc = mx_all[:, slot:slot + 1]
                nc.vector.tensor_reduce(mxc, mx4[:], AX.X, ALU.max)
                nc.vector.tensor_scalar_max(mxc, mxc, 1e-5)
                r127 = small.tile([P, 1], dt.float32, tag="r127", name="r127")
                nc.vector.reciprocal(r127[:], mxc)
                nc.vector.tensor_scalar_mul(r127[:], r127[:], 127.0)
                # n2 = clip(round(a * 127/max), -128, 127), in-place round
                nc.vector.tensor_scalar(asl[:], asl[:], r127[:], MAGIC,
                                        ALU.mult, ALU.add)
                nc.vector.tensor_scalar(asl[:], asl[:], MAGIC, -128.0,
                                        ALU.subtract, ALU.max)
                n2 = work.tile([P, F_], dt.bfloat16, tag="n2", name="n2",
                               bufs=3)
                nc.vector.tensor_scalar(n2[:], asl[:], 127.0, None, ALU.min)
                return n2

            def trans(n2):
                """Tiled transpose of the quantized fc1 output via the DMA
                xbar: n2T[p, k, t] = n2[t, k*128+p]. Runs on the DMA engines,
                so the PE does matmuls only."""
                n2T = work.tile([P, KF, P], dt.bfloat16, tag="n2T",
                                name="n2T")
                nc.sync.dma_start_transpose(n2T[:], n2[:])
                return n2T

            def fc2(n2T, w2t, out_d, mt, last=False):
                """fc2; raw accumulators are copied out on the DVE and the
                per-token scale is applied on the host, keeping the
                end-of-kernel tail short. The last tile tapers its output
                chunks (512/384/128) so the final copy+store is small."""
                outsb = work.tile([P, D_], dt.float32, tag="outsb",
                                  name="outsb")
                chunks = [(0, 512), (512, 384), (896, 128)] if last else [
                    (0, 512), (512, 512)]
                for c0, cw in chunks:
                    ps2 = pp2.tile([P, 512], dt.float32, tag="ps2", name="ps2")
                    for k in range(KF):
                        nc.tensor.matmul(
                            ps2[:, 0:cw],
                            n2T[:, k, :],
                            w2t[:, k, c0:c0 + cw],
                            start=(k == 0), stop=(k == KF - 1))
                    nc.scalar.copy(outsb[:, c0:c0 + cw], ps2[:, 0:cw])
                    nc.sync.dma_start(
                        out_d[mt * P:(mt + 1) * P, c0:c0 + cw],
                        outsb[:, c0:c0 + cw])

            # 3-stage software pipeline: per iteration the PE runs
            # fc1(i) then fc2(i-2), with tile i-1's DVE quant chain and
            # xbar-transpose DMA hidden under that matmul work, including
            # at the drain.
            trans_q = []   # awaiting transpose
            fc2_q = []     # awaiting fc2
            for i, (at, w1t, w2t, cs1_c, slot, out_d, mt) in enumerate(tiles):
                n2 = front(at, w1t, cs1_c, slot, mt)
                trans_q.append((n2, w2t, out_d, mt))
                if len(trans_q) > 1:
                    tn2, tw2t, tout_d, tmt = trans_q.pop(0)
                    fc2_q.append((trans(tn2), tw2t, tout_d, tmt))
                if len(fc2_q) > 1:
                    fc2(*fc2_q.pop(0))
            # ship the per-token absmaxes once the last front has run
            nc.sync.dma_start(out_v[:], mx_all[:])
            while trans_q:
                tn2, tw2t, tout_d, tmt = trans_q.pop(0)
                fc2_q.append((trans(tn2), tw2t, tout_d, tmt))
                if fc2_q:
                    fc2(*fc2_q.pop(0))
            while fc2_q:
                p = fc2_q.pop(0)
                fc2(*p, last=not fc2_q)

    nc.compile()
    return nc


def _get_nc():
    global _NC_CACHE
    if _NC_CACHE is None:
        _NC_CACHE = _build_nc()
    return _NC_CACHE


# ----------------------------------------------------------------------------
# entry point
# ----------------------------------------------------------------------------

def _prepare(x, rms_w, w1_shared, w2_shared, w1_routed, w2_routed, router_w,
             top_k):
    x = np.asarray(x)
    B, S, D = x.shape
    T = B * S
    E = np.asarray(router_w).shape[0]
    SH = np.asarray(w1_shared).shape[0]
    k_ = int(top_k)
    assert (T, D, E, SH) == (T_, D_, E_, 2) and k_ == 2

    h = _rmsnorm(x.reshape(T, D).astype(np.float32), np.asarray(rms_w))
    n1, s1 = _quant_a(h)
    idx, g = _route(h, np.asarray(router_w), k_)

    # ternary weights + scales
    t1r, sc1r, t2r, sc2r = [], [], [], []
    for e in range(E):
        t, s = _quant_w(np.asarray(w1_routed)[e]); t1r.append(t); sc1r.append(s)
        t, s = _quant_w(np.asarray(w2_routed)[e]); t2r.append(t); sc2r.append(s)
    t1s, sc1s_, t2s, sc2s_ = [], [], [], []
    for e in range(SH):
        t, s = _quant_w(np.asarray(w1_shared)[e]); t1s.append(t); sc1s_.append(s)
        t, s = _quant_w(np.asarray(w2_shared)[e]); t2s.append(t); sc2s_.append(s)

    n1_bf = n1.astype(BF16)

    # dispatch: token lists per expert (ascending order)
    tok_lists = [np.where((idx == e).any(axis=1))[0] for e in range(E)]
    gate_of = np.zeros((T, E), dtype=np.float32)
    for slot in range(k_):
        gate_of[np.arange(T), idx[:, slot]] += g[:, slot]

    def pack_act(rows):
        # [T, D] -> [mt, d_sub, k, t] -> [(mt d_sub), (k t)]
        mt = rows.shape[0] // P
        return np.ascontiguousarray(
            rows.reshape(mt, P, D_ // P, P).transpose(0, 3, 2, 1)
                .reshape(mt * P, D_))

    in_maps = []
    for i in range(NCORES):
        toks = tok_lists[i][:C_ROUT]
        nct = len(toks)
        a_r = np.zeros((C_ROUT, D_), dtype=BF16)
        a_r[:nct] = n1_bf[toks]
        cs1_v = np.zeros(C_ROUT, dtype=np.float32)
        cs1_v[:nct] = sc1r[i] / s1[toks]

        sh, blk = i // 4, i % 4
        btok = slice(blk * T_SH, (blk + 1) * T_SH)
        a_s = n1_bf[btok]
        cs1s_v = (sc1s_[sh] / s1[btok]).astype(np.float32)

        in_maps.append({
            "a_r": pack_act(a_r),
            "a_s": pack_act(np.ascontiguousarray(a_s)),
            "w1r": t1r[i].astype(F8),
            "w2r": t2r[i].astype(F8),
            "w1s": t1s[sh].astype(F8),
            "w2s": t2s[sh].astype(F8),
            "sc_r": np.ascontiguousarray(cs1_v.reshape(-1, P).T),
            "sc_s": np.ascontiguousarray(cs1s_v.reshape(-1, P).T),
        })

    meta = {
        "B": B, "S": S, "T": T,
        "tok_lists": tok_lists, "gate_of": gate_of,
        "n1": n1, "s1": s1, "t1r": t1r, "sc1r": sc1r,
        "t2r": t2r, "sc2r": sc2r, "sc2s": sc2s_,
    }
    return in_maps, meta


def _assemble(results, meta):
    T = meta["T"]
    tok_lists = meta["tok_lists"]
    MTR = C_ROUT // P
    acc = np.zeros((T, D_), dtype=np.float32)
    for i in range(NCORES):
        om = results[i]
        sh, blk = i // 4, i % 4
        # out_v columns: routed tiles then shared tiles; token = mt*P + p
        mx = om["out_v"]
        mx_r = mx[:, :MTR].T.reshape(-1)
        mx_s = mx[:, MTR:].T.reshape(-1)
        toks = tok_lists[i][:C_ROUT]
        nct = len(toks)
        v_r = (meta["gate_of"][toks, i] * meta["sc2r"][i] / F32(127.0)
               * mx_r[:nct]).astype(np.float32)
        np.add.at(acc, toks, om["out_r"][:nct] * v_r[:, None])
        v_s = (meta["sc2s"][sh] / F32(127.0) * mx_s).astype(np.float32)
        acc[blk * T_SH:(blk + 1) * T_SH] += om["out_s"] * v_s[:, None]
        # capacity-overflow fallback (exact replication on host)
        if len(tok_lists[i]) > C_ROUT:
            extra = tok_lists[i][C_ROUT:]
            out_e = _expert_mlp_rows(
                meta["n1"][extra], meta["s1"][extra], meta["t1r"][i],
                meta["sc1r"][i], meta["t2r"][i], meta["sc2r"][i])
            acc[extra] += meta["gate_of"][extra, i][:, None] * out_e
    return acc.reshape(meta["B"], meta["S"], D_).astype(np.float32)


def kernel(x, rms_w, w1_shared, w2_shared, w1_routed, w2_routed, router_w,
           top_k):
    global _LAST_RESULTS
    in_maps, meta = _prepare(x, rms_w, w1_shared, w2_shared, w1_routed,
                             w2_routed, router_w, top_k)
    from concourse import bass_utils
    nc = _get_nc()
    res = bass_utils.run_bass_kernel_spmd(
        nc, in_maps, core_ids=list(range(NCORES)), trace=TRACE)
    _LAST_RESULTS = res
    return _assemble(res.results, meta)

